# revision 7
# baseline (speedup 1.0000x reference)
"""Trainium2 Bass kernel for nn_AttentionV1 (spatial attention block).

Reference computation (per batch b):
    q = wq @ x + bq            [128, 4096]
    k = wk @ aux + bk          [128, 4096]
    v = wv @ x + bv            [128, 4096]
    s = k^T q                  [4096 k, 4096 q]
    a = softmax(s, axis=q)     (normalize across QUERIES for each key row)
    y = v @ a                  [128, 4096]
    z = wz @ y + bz + x        [256, 4096]

Sharding: 8 cores = 4 batches x 2 query-halves.  Each core owns 2048 query
columns of one batch and computes K / V^T for all 4096 keys.  The softmax
normalization axis (q) is sharded, so per 512-key chunk the two cores of a
pair AllReduce their partial exp-rowsums (a 4 KB message) and fold 1/rowsum
into the V^T rows before the y matmuls.  y accumulates in PSUM across all
8 key chunks; the output projection + residual is computed per query half,
so no large tensors ever cross cores.
"""

import sys

if "/opt/trn_rl_repo" not in sys.path:
    sys.path.insert(0, "/opt/trn_rl_repo")

import numpy as np

import concourse.bass as bass  # noqa: F401  (import keeps bass registered)
import concourse.mybir as mybir
import concourse.tile as tile
from concourse import bacc
from concourse import bass2jax

F32 = mybir.dt.float32
F32R = mybir.dt.float32r
AF = mybir.ActivationFunctionType
ALU = mybir.AluOpType

# Problem constants (hardcoded per harness contract).
B, C = 4, 256
CH = 128          # C // 2, the qkv channel count == SBUF partition count
N = 4096          # H * W
NQ = 2048         # query columns per core (N / 2)
NCORES = 8
NCHUNK = 8        # key chunks
NSUB = 4          # 128-row subchunks per key chunk
QT = 512          # matmul moving-dim tile
EXP_BIAS = -40.0  # constant shift inside exp() to avoid fp32 overflow

# Matmul precision knobs: float32r runs the PE at 4x the fp32 rate for
# 512-wide moving operands.  Attention = the S and y matmuls; proj = Q/K/z.
F32R_ATTN = True
F32R_PROJ = True

GROUPS = [[0, 1], [2, 3], [4, 5], [6, 7]]


def build_program(f32r_attn: bool = F32R_ATTN, f32r_proj: bool = F32R_PROJ):
    nc = bacc.Bacc("TRN2", target_bir_lowering=False, debug=False,
                   num_devices=NCORES)

    DTA = F32R if f32r_attn else F32   # attention-path matmul dtype
    DTP = F32R if f32r_proj else F32   # projection-path matmul dtype

    x_d = nc.dram_tensor("x", [C, N], F32, kind="ExternalInput")
    xq_d = nc.dram_tensor("xq", [C, NQ], DTP, kind="ExternalInput")
    aux_d = nc.dram_tensor("aux", [C, N], DTP, kind="ExternalInput")
    wqT_d = nc.dram_tensor("wqT", [C, CH], DTP, kind="ExternalInput")
    wkT_d = nc.dram_tensor("wkT", [C, CH], DTP, kind="ExternalInput")
    wvT_d = nc.dram_tensor("wvT", [C, CH], F32, kind="ExternalInput")
    wzT_d = nc.dram_tensor("wzT", [CH, C], DTA, kind="ExternalInput")
    bq_d = nc.dram_tensor("bq", [CH, 1], F32, kind="ExternalInput")
    bk_d = nc.dram_tensor("bk", [CH, 1], F32, kind="ExternalInput")
    bv_d = nc.dram_tensor("bv", [1, CH], F32, kind="ExternalInput")
    bz_d = nc.dram_tensor("bz", [C, 1], F32, kind="ExternalInput")
    ones_d = nc.dram_tensor("ones", [1, CH], F32, kind="ExternalInput")
    z_d = nc.dram_tensor("z", [C, NQ], F32, kind="ExternalOutput")


    with tile.TileContext(nc) as tc:
        with (
            tc.tile_pool(name="const", bufs=1) as constp,
            tc.tile_pool(name="persist", bufs=1) as persist,
            tc.tile_pool(name="dram", bufs=NCHUNK, space="DRAM") as dramp,
        ):
            # ---- constants ----
            wqT = [constp.tile([128, CH], DTP, tag=f"wq{i}", name=f"wq{i}") for i in range(2)]
            wkT = [constp.tile([128, CH], DTP, tag=f"wk{i}", name=f"wk{i}") for i in range(2)]
            wvT = [constp.tile([128, CH], F32, tag=f"wv{i}", name=f"wv{i}") for i in range(2)]
            for i in range(2):
                rsl = slice(i * 128, (i + 1) * 128)
                nc.sync.dma_start(wqT[i][:], wqT_d[rsl, :])
                nc.sync.dma_start(wkT[i][:], wkT_d[rsl, :])
                nc.sync.dma_start(wvT[i][:], wvT_d[rsl, :])
            wzT_sb = constp.tile([128, C], DTA, tag="wz", name="wzT_sb")
            nc.sync.dma_start(wzT_sb[:], wzT_d[:, :])
            bq_sb = constp.tile([CH, 1], F32, tag="bq", name="bq_sb")
            bk_sb = constp.tile([CH, 1], F32, tag="bk", name="bk_sb")
            nc.sync.dma_start(bq_sb[:], bq_d[:, :])
            nc.sync.dma_start(bk_sb[:], bk_d[:, :])
            bz_sb = [constp.tile([128, 1], F32, tag=f"bz{i}", name=f"bz{i}") for i in range(2)]
            for i in range(2):
                nc.sync.dma_start(bz_sb[i][:], bz_d[i * 128:(i + 1) * 128, :])
            bv_row = constp.tile([1, CH], F32, tag="bv", name="bv_row")
            nc.sync.dma_start(bv_row[:], bv_d[:, :])
            ones_row = constp.tile([1, CH], F32, tag="ones", name="ones_row")
            nc.sync.dma_start(ones_row[:], ones_d[:, :])
            ebias = constp.tile([128, 1], F32, tag="ebias", name="ebias")
            nc.vector.memset(ebias[:], EXP_BIAS)

            # ---- persistent activations ----
            xq_sb = [persist.tile([128, NQ], DTP, tag=f"xq{i}", name=f"xq{i}") for i in range(2)]
            for i in range(2):
                nc.sync.dma_start(xq_sb[i][:], xq_d[i * 128:(i + 1) * 128, :])
            K_sb = persist.tile([128, N], DTA, tag="K", name="K_sb")
            Q_sb = persist.tile([128, NQ], DTA, tag="Q", name="Q_sb")
            Vt = [persist.tile([128, CH], DTA, tag=f"vt{g}", name=f"vt{g}") for g in range(32)]
            y_sb = [persist.tile([128, QT], DTA, tag=f"y{qt}", name=f"ysb{qt}") for qt in range(4)]

            # ---- projections: K (full keys), Q (local queries), V^T ----
            with (
                tc.tile_pool(name="xaux", bufs=1) as xauxp,
                tc.tile_pool(name="pj_ps", bufs=4, space="PSUM") as pjps,
            ):
                x_sb = [xauxp.tile([128, N], F32, tag=f"x{i}", name=f"x{i}") for i in range(2)]
                aux_sb = [xauxp.tile([128, N], DTP, tag=f"a{i}", name=f"a{i}") for i in range(2)]
                for i in range(2):
                    rsl = slice(i * 128, (i + 1) * 128)
                    nc.sync.dma_start(x_sb[i][:], x_d[rsl, :])
                    nc.sync.dma_start(aux_sb[i][:], aux_d[rsl, :])

                # K = wk^T.T @ aux + bk, in 512-column tiles
                for t in range(N // QT):
                    ps = pjps.tile([128, QT], F32, tag="pj", name="pjps")
                    sl = slice(t * QT, (t + 1) * QT)
                    nc.tensor.matmul(ps[:], wkT[0][:],
                                     aux_sb[0][:, sl],
                                     start=True, stop=False)
                    nc.tensor.matmul(ps[:], wkT[1][:],
                                     aux_sb[1][:, sl],
                                     start=False, stop=True)
                    nc.vector.tensor_scalar_add(K_sb[:, sl], ps[:], bk_sb[:])

                # Q = wq^T.T @ xq + bq
                for t in range(NQ // QT):
                    ps = pjps.tile([128, QT], F32, tag="pj", name="pjps")
                    sl = slice(t * QT, (t + 1) * QT)
                    nc.tensor.matmul(ps[:], wqT[0][:],
                                     xq_sb[0][:, sl],
                                     start=True, stop=False)
                    nc.tensor.matmul(ps[:], wqT[1][:],
                                     xq_sb[1][:, sl],
                                     start=False, stop=True)
                    nc.vector.tensor_scalar_add(Q_sb[:, sl], ps[:], bq_sb[:])

                # V^T[k, c] = x[:, k]^T @ wv^T + bv  (bias via rank-1 matmul)
                for g in range(32):
                    ps = pjps.tile([128, CH], F32, tag="pjv", name="pjvps")
                    sl = slice(g * 128, (g + 1) * 128)
                    nc.tensor.matmul(ps[:], x_sb[0][:, sl], wvT[0][:],
                                     start=True, stop=False)
                    nc.tensor.matmul(ps[:], x_sb[1][:, sl], wvT[1][:],
                                     start=False, stop=False)
                    nc.tensor.matmul(ps[:], ones_row[:], bv_row[:],
                                     start=False, stop=True)
                    nc.vector.tensor_copy(Vt[g][:], ps[:])

            # ---- stage 1: attention ----
            with (
                tc.tile_pool(name="E", bufs=10) as Ep,
                tc.tile_pool(name="rp", bufs=3) as rp,
                tc.tile_pool(name="s_ps", bufs=2, space="PSUM") as sps,
                tc.tile_pool(name="y_ps", bufs=4, space="PSUM") as yps,
            ):
                y_ps = [yps.tile([128, QT], F32, tag="y", name="yps") for _ in range(4)]
                for kc in range(NCHUNK):
                    E = [Ep.tile([128, NQ], DTA, tag="E", name="Etile") for _ in range(NSUB)]
                    r_part = rp.tile([128, 2 * NSUB], F32, tag="rpart", name="rpart")
                    for s in range(NSUB):
                        ksl = slice((kc * NSUB + s) * 128,
                                    (kc * NSUB + s + 1) * 128)
                        for st in range(2):
                            ps = sps.tile([128, 1024], F32, tag="s")
                            for hh in range(2):
                                qsl = slice(st * 1024 + hh * QT,
                                            st * 1024 + (hh + 1) * QT)
                                nc.tensor.matmul(
                                    ps[:, hh * QT:(hh + 1) * QT],
                                    K_sb[:, ksl],
                                    Q_sb[:, qsl],
                                    start=True, stop=True)
                            # e = exp(s + EXP_BIAS); accum_out = rowsum(e)
                            nc.scalar.activation(
                                E[s][:, st * 1024:(st + 1) * 1024], ps[:],
                                AF.Exp, bias=ebias[:], scale=1.0,
                                accum_out=r_part[:, st * NSUB + s:
                                                 st * NSUB + s + 1])
                    # complete the rowsums across the query-half pair
                    rin = dramp.tile([128, 2 * NSUB], F32, tag="rin", name="rin")
                    rout = dramp.tile([128, 2 * NSUB], F32, tag="rout", name="rout")
                    nc.sync.dma_start(rin[:], r_part[:])
                    nc.gpsimd.collective_compute(
                        "AllReduce", ALU.add, replica_groups=GROUPS,
                        ins=[rin.opt()], outs=[rout.opt()])
                    r_red = rp.tile([128, 2 * NSUB], F32, tag="rred", name="rred")
                    nc.sync.dma_start(r_red[:], rout[:])
                    rinv = rp.tile([128, NSUB], F32, tag="rinv", name="rinv")
                    nc.vector.tensor_add(rinv[:], r_red[:, 0:NSUB],
                                         r_red[:, NSUB:2 * NSUB])
                    nc.vector.reciprocal(rinv[:], rinv[:])
                    for s in range(NSUB):
                        g = kc * NSUB + s
                        nc.vector.tensor_scalar_mul(Vt[g][:], Vt[g][:],
                                                    rinv[:, s:s + 1])
                    # y += (V^T/r).T @ E, accumulated in PSUM across chunks
                    for qt in range(4):
                        qsl = slice(qt * QT, (qt + 1) * QT)
                        for s in range(NSUB):
                            nc.tensor.matmul(
                                y_ps[qt][:],
                                Vt[kc * NSUB + s][:],
                                E[s][:, qsl],
                                start=(kc == 0 and s == 0),
                                stop=(kc == NCHUNK - 1 and s == NSUB - 1))
                for qt in range(4):
                    nc.vector.tensor_copy(y_sb[qt][:], y_ps[qt][:])

            # ---- output projection + residual ----
            with (
                tc.tile_pool(name="z_ps", bufs=4, space="PSUM") as zps,
                tc.tile_pool(name="zt", bufs=4) as ztp,
            ):
                for qt in range(4):
                    qsl = slice(qt * QT, (qt + 1) * QT)
                    for co in range(2):
                        ps = zps.tile([128, QT], F32, tag="z", name="zps")
                        nc.tensor.matmul(
                            ps[:],
                            wzT_sb[:, co * 128:(co + 1) * 128],
                            y_sb[qt][:],
                            start=True, stop=True)
                        zt = ztp.tile([128, QT], F32, tag="zt", name="zt")
                        # z = (psum + bz) + xq
                        nc.vector.scalar_tensor_tensor(
                            zt[:], ps[:], bz_sb[co][:],
                            xq_sb[co][:, qsl].bitcast(F32),
                            op0=ALU.add, op1=ALU.add)
                        nc.sync.dma_start(
                            z_d[co * 128:(co + 1) * 128, qsl], zt[:])

    nc.compile()
    return nc


def make_in_maps(inputs: dict) -> list:
    x = np.ascontiguousarray(np.asarray(inputs["x"], np.float32)
                             .reshape(B, C, N))
    aux = np.ascontiguousarray(np.asarray(inputs["aux"], np.float32)
                               .reshape(B, C, N))
    wqT = np.ascontiguousarray(np.asarray(inputs["wq_w"], np.float32).T)
    wkT = np.ascontiguousarray(np.asarray(inputs["wk_w"], np.float32).T)
    wvT = np.ascontiguousarray(np.asarray(inputs["wv_w"], np.float32).T)
    wzT = np.ascontiguousarray(np.asarray(inputs["wz_w"], np.float32).T)
    bq = np.asarray(inputs["wq_b"], np.float32).reshape(CH, 1)
    bk = np.asarray(inputs["wk_b"], np.float32).reshape(CH, 1)
    bv = np.asarray(inputs["wv_b"], np.float32).reshape(1, CH)
    bz = np.asarray(inputs["wz_b"], np.float32).reshape(C, 1)
    ones = np.ones((1, CH), np.float32)
    in_maps = []
    for c in range(NCORES):
        b, h = c // 2, c % 2
        in_maps.append({
            "x": x[b],
            "xq": np.ascontiguousarray(x[b][:, h * NQ:(h + 1) * NQ]),
            "aux": aux[b],
            "wqT": wqT, "wkT": wkT, "wvT": wvT, "wzT": wzT,
            "bq": bq, "bk": bk, "bv": bv, "bz": bz, "ones": ones,
        })
    return in_maps


class Runner:
    """Compile once, then run the SPMD kernel any number of times.

    Mirrors bass2jax.run_bass_via_pjrt's multi-core branch but keeps the
    jitted executable so repeated calls don't re-trace/re-compile.
    """

    def __init__(self, f32r_attn: bool = F32R_ATTN,
                 f32r_proj: bool = F32R_PROJ):
        import jax
        from jax.experimental.shard_map import shard_map
        from jax.sharding import Mesh, PartitionSpec

        self.nc = build_program(f32r_attn, f32r_proj)
        bass2jax.install_neuronx_cc_hook()
        nc = self.nc
        assert nc.dbg_addr is None
        partition_name = (nc.partition_id_tensor.name
                          if nc.partition_id_tensor else None)

        in_names, out_names, out_avals, zero_outs = [], [], [], []
        for alloc in nc.m.functions[0].allocations:
            if not isinstance(alloc, mybir.MemoryLocationSet):
                continue
            name = alloc.memorylocations[0].name
            if alloc.kind == "ExternalInput":
                if name != partition_name:
                    in_names.append(name)
            elif alloc.kind == "ExternalOutput":
                out_names.append(name)
                shape = tuple(alloc.tensor_shape)
                dtype = mybir.dt.np(alloc.dtype)
                out_avals.append(jax.core.ShapedArray(shape, dtype))
                zero_outs.append(np.zeros(shape, dtype))
        self.in_names = list(in_names)
        self.out_names = out_names
        self.out_avals = out_avals
        n_params = len(in_names)
        n_outs = len(out_avals)
        all_names = in_names + out_names
        if partition_name is not None:
            all_names = all_names + [partition_name]

        def _body(*args):
            operands = list(args)
            if partition_name is not None:
                operands.append(bass2jax.partition_id_tensor())
            outs = bass2jax._bass_exec_p.bind(
                *operands,
                out_avals=tuple(out_avals),
                in_names=tuple(all_names),
                out_names=tuple(out_names),
                lowering_input_output_aliases=(),
                sim_require_finite=True,
                sim_require_nnan=True,
                nc=nc,
            )
            return tuple(outs)

        devices = jax.devices()[:NCORES]
        mesh = Mesh(np.asarray(devices), ("core",))
        in_specs = (PartitionSpec("core"),) * (n_params + n_outs)
        out_specs = (PartitionSpec("core"),) * n_outs
        self._sharded = jax.jit(
            shard_map(_body, mesh=mesh, in_specs=in_specs,
                      out_specs=out_specs, check_rep=False),
            donate_argnums=tuple(range(n_params, n_params + n_outs)),
            keep_unused=True,
        )
        self._zero_outs = zero_outs

    def run(self, in_maps):
        concat_in = [
            np.concatenate([np.asarray(in_maps[c][name])
                            for c in range(NCORES)], axis=0)
            for name in self.in_names
        ]
        concat_zeros = [
            np.zeros((NCORES * z.shape[0], *z.shape[1:]), z.dtype)
            for z in self._zero_outs
        ]
        out_arrs = self._sharded(*concat_in, *concat_zeros)
        return [
            {
                name: np.asarray(out_arrs[i]).reshape(
                    NCORES, *self.out_avals[i].shape)[c]
                for i, name in enumerate(self.out_names)
            }
            for c in range(NCORES)
        ]


_RUNNER = None


def get_runner() -> Runner:
    global _RUNNER
    if _RUNNER is None:
        _RUNNER = Runner()
    return _RUNNER


def assemble(results) -> np.ndarray:
    out = np.empty((B, C, N), np.float32)
    for c in range(NCORES):
        b, h = c // 2, c % 2
        out[b][:, h * NQ:(h + 1) * NQ] = results[c]["z"]
    return out.reshape(B, C, 64, 64)


def kernel(**inputs) -> np.ndarray:
    runner = get_runner()
    results = runner.run(make_in_maps(inputs))
    return assemble(results)
